# revision 26
# baseline (speedup 1.0000x reference)
"""Causal multi-head attention (B=4, L=2048, D=1024, H=16) on 8 TRN2 NeuronCores.

Sharding: core = (batch b, head-group hg), b in 0..3, hg in 0..1; each core
computes its batch x 8 heads and the partial out-projection; host sums the two
head-group partials per batch.

Fully-pipelined single-pass structure (vs. the phase-sequential baseline):
  - all operands bf16 (PSUM accumulation f32), which halves SBUF so the
    x / W tiles stay resident and projection matmuls interleave with
    attention as PE "filler" jobs between score tiles:
      hp0 carries the v projection + hp1's q/k projection,
      hp1/hp2 carry hp+1's q/k projection,
      hp3 carries the out-projection of finished query blocks.
  - scores -> exp in 1024-col PSUM tiles (two banks) to halve ScalarE
    instruction-start overhead; exp runs concurrently with attn@v and the
    interleaved projections.
  - softmax denominator via ones-column in v_aug; reciprocal on DVE, then
    gpsimd partition_broadcast (Pool engine) replaces the baseline's
    DRAM round-trip; odd heads use a [ones, v] column order so their
    attn@v output lands on partitions 63:128 and normalizes in place.

kernel(**inputs) takes the full unsharded f32 inputs and returns the full
f32 output.
"""

import numpy as np
from ml_dtypes import bfloat16

import concourse.bass as bass
import concourse.mybir as mybir
import concourse.tile as tile
from concourse import bacc
from concourse.bass_utils import run_bass_kernel_spmd

F32 = mybir.dt.float32
BF16 = mybir.dt.bfloat16

L = 2048          # sequence length
D = 1024          # model dim
HG = 8            # heads per core
DH = 64           # head dim
DHG = HG * DH     # 512, head-group width
DC = D // 128     # 8 contraction chunks for projections
LT = L // 128     # 16 key-position chunks
QB = L // 512     # 4 query blocks of 512
N_CORES = 8


# column offset of kp-chunk j's storage inside the packed causal attnT buffer
def _off(j):
    return 2048 * j - 128 * (j * (j - 1) // 2)


ATT_W = _off(LT)  # 17408 packed causal columns per head


SKIP = set()


def build_kernel(reps: int = 0, phases: str = "pao"):
    """Build the SPMD Bass program. reps>0 wraps the body in a hardware loop
    (body executed reps+1 times total) for timing."""
    nc = bacc.Bacc()

    xT = nc.dram_tensor("xT", [D, L], BF16, kind="ExternalInput")
    wqT = nc.dram_tensor("wqT", [D, DHG], BF16, kind="ExternalInput")
    wkT = nc.dram_tensor("wkT", [D, DHG], BF16, kind="ExternalInput")
    wvT = nc.dram_tensor("wvT", [D, DHG], BF16, kind="ExternalInput")
    woT = nc.dram_tensor("woT", [DHG, D], BF16, kind="ExternalInput")
    out = nc.dram_tensor("out", [L, D], BF16, kind="ExternalOutput")

    xT_r = xT[:, :].rearrange("(c p) l -> p c l", p=128)
    wqT_r = wqT[:, :].rearrange("(c p) m -> p c m", p=128)
    wkT_r = wkT[:, :].rearrange("(c p) m -> p c m", p=128)
    wvT_r = wvT[:, :].rearrange("(c p) m -> p c m", p=128)
    woT_r = woT[:, :].rearrange("(c p) n -> p c n", p=128)
    out_r = out[:, :].rearrange("(t p) n -> p t n", p=128)

    with tile.TileContext(nc) as tc:
        ctx_body(nc, tc, xT_r, wqT_r, wkT_r, wvT_r, woT_r, out_r, reps, phases)
    nc.compile()
    return nc


def ctx_body(nc, tc, xT_r, wqT_r, wkT_r, wvT_r, woT_r, out_r, reps, phases="pao"):
    from contextlib import ExitStack

    with ExitStack() as es:
        persist = es.enter_context(tc.tile_pool(name="persist", bufs=1))
        mask_sb = persist.tile([128, 128], BF16)  # upper-tri (incl diag) ones
        ones_sb = persist.tile([128, 128], BF16)  # rank-1 denom broadcast
        nc.vector.memset(ones_sb, 1.0)

        # constant setup (outside the timing loop)
        # mask[kp, qp] = 1 where kp <= qp else 0
        nc.gpsimd.memset(mask_sb, 1.0)
        nc.gpsimd.affine_select(
            out=mask_sb,
            in_=mask_sb,
            compare_op=mybir.AluOpType.is_ge,
            fill=0.0,
            base=0,
            pattern=[[1, 128]],
            channel_multiplier=-1,
        )

        def body():
            with ExitStack() as bs:
                glob = bs.enter_context(tc.tile_pool(name="glob", bufs=1))
                qkp = bs.enter_context(tc.tile_pool(name="qkp", bufs=2))
                att = bs.enter_context(tc.tile_pool(name="att", bufs=2))
                nrm = bs.enter_context(tc.tile_pool(name="nrm", bufs=1))
                oev = bs.enter_context(tc.tile_pool(name="oev", bufs=1))
                wkps = bs.enter_context(
                    tc.tile_pool(name="wkps", bufs=1, space="PSUM")
                )

                xT_sb = glob.tile([128, DC, L], BF16)
                wq_sb = glob.tile([128, DC, DHG], BF16)
                wk_sb = glob.tile([128, DC, DHG], BF16)
                wv_sb = glob.tile([128, DC, DHG], BF16)
                wo_sb = glob.tile([128, 4, D], BF16)
                v_sb = glob.tile([128, LT, HG, DH + 1], BF16)
                outT_sb = glob.tile([128, 4, L], BF16)

                # load order tracks first use: x + head-pair-0 slices of
                # wq/wk gate the prologue, the rest trickles in behind
                for c in range(DC):
                    nc.sync.dma_start(out=xT_sb[:, c, :], in_=xT_r[:, c, :])
                    nc.sync.dma_start(
                        out=wq_sb[:, c, 0:128], in_=wqT_r[:, c, 0:128]
                    )
                    nc.sync.dma_start(
                        out=wk_sb[:, c, 0:128], in_=wkT_r[:, c, 0:128]
                    )
                for c in range(DC):
                    nc.sync.dma_start(
                        out=wq_sb[:, c, 128:DHG], in_=wqT_r[:, c, 128:DHG]
                    )
                    nc.sync.dma_start(
                        out=wk_sb[:, c, 128:DHG], in_=wkT_r[:, c, 128:DHG]
                    )
                for c in range(DC):
                    nc.sync.dma_start(out=wv_sb[:, c, :], in_=wvT_r[:, c, :])
                nc.sync.dma_start(out=wo_sb, in_=woT_r)

                # ones column of v_aug (denominator row of attn@v output)
                nc.vector.memset(v_sb[:, :, :, DH : DH + 1], 1.0)

                def pj_one(w_sb, t, qb, dst):
                    # one projection (q or k) for head-pair t, query block qb
                    ps = wkps.tile([128, 512], F32, tag="w5", bufs=3)
                    for c in range(DC):
                        nc.tensor.matmul(
                            ps,
                            w_sb[:, c, t * 128 : (t + 1) * 128],
                            xT_sb[:, c, qb * 512 : (qb + 1) * 512],
                            start=(c == 0),
                            stop=(c == DC - 1),
                        )
                    nc.vector.tensor_copy(dst[:, qb * 512 : (qb + 1) * 512], ps)

                def pj_qk(t, qb, dq, dk):
                    pj_one(wq_sb, t, qb, dq)
                    pj_one(wk_sb, t, qb, dk)

                def pj_v(it):
                    # v projection for kp chunk it (all 8 heads)
                    ps = wkps.tile([128, 512], F32, tag="w5", bufs=3)
                    for c in range(DC):
                        nc.tensor.matmul(
                            ps,
                            xT_sb[:, c, it * 128 : (it + 1) * 128],
                            wv_sb[:, c, :],
                            start=(c == 0),
                            stop=(c == DC - 1),
                        )
                    nc.vector.tensor_copy(
                        v_sb[:, it, :, 0:DH],
                        ps.rearrange("p (h d) -> p h d", h=HG),
                    )

                def op_qt(qt):
                    # out-projection partial for query tile qt; two 512-wide
                    # PSUM tiles from the deeper w5 ring (the 2-deep wk ring
                    # is busy with score tiles when this interleaves into hp3)
                    ot = oev.tile([128, D], BF16, tag="ot", bufs=2)
                    for nh in range(2):
                        ps = wkps.tile([128, 512], F32, tag="w5", bufs=3)
                        for c in range(4):
                            nc.tensor.matmul(
                                ps,
                                outT_sb[:, c, qt * 128 : (qt + 1) * 128],
                                wo_sb[:, c, nh * 512 : (nh + 1) * 512],
                                start=(c == 0),
                                stop=(c == 3),
                            )
                        nc.vector.tensor_copy(
                            ot[:, nh * 512 : (nh + 1) * 512], ps
                        )
                    nc.sync.dma_start(out=out_r[:, qt, :], in_=ot)

                def sc_group(hp, b, atl, cq, ck, fillers):
                    # scores + exp for j-group 4b..4b+3, fillers interleaved
                    fillers = list(fillers)
                    for j in range(4 * b, 4 * b + 4):
                        ncols = L - 128 * j
                        for hh in () if "sc" in SKIP else range(2):
                            p0 = hh * 64
                            for c0 in range(0, ncols, 1024):
                                w = min(1024, ncols - c0)
                                ps = wkps.tile([128, 1024], F32, tag="wk", bufs=2)
                                for s0 in range(0, w, 512):
                                    sw = min(512, w - s0)
                                    q0 = 128 * j + c0 + s0
                                    nc.tensor.matmul(
                                        ps[:, s0 : s0 + sw],
                                        ck[p0 : p0 + 64, j * 128 : (j + 1) * 128],
                                        cq[p0 : p0 + 64, q0 : q0 + sw],
                                        start=True,
                                        stop=True,
                                    )
                                if "exp" not in SKIP:
                                    nc.scalar.activation(
                                        atl[hh][:, _off(j) + c0 : _off(j) + c0 + w],
                                        ps[:, :w],
                                        mybir.ActivationFunctionType.Exp,
                                        scale=0.125,
                                    )
                            if "exp" not in SKIP:
                                # mask the diagonal block of this j on the
                                # (otherwise idle) Pool engine so it never
                                # queues behind DVE work on the attn@v path
                                nc.gpsimd.tensor_mul(
                                    atl[hh][:, _off(j) : _off(j) + 128],
                                    atl[hh][:, _off(j) : _off(j) + 128],
                                    mask_sb,
                                )
                        if fillers:
                            fillers.pop(0)()
                    for f in fillers:
                        f()

                def av_group(hp, b, atl):
                    # attn @ v_aug -> outT for qp-block b, both heads.
                    # Normalization: reciprocal of the denominator row (DVE,
                    # bf16), broadcast across partitions via a rank-1 bf16
                    # matmul (ones^T x recip -> PSUM), multiply the evacuated
                    # attn@v rows by it. Odd head's rows are DMA-shifted to
                    # partitions 64:128 before the multiply so it writes outT
                    # in place.
                    if "av" in SKIP:
                        return
                    pss, recips, usts = [], [], []
                    jmax = 4 * b + 3
                    for hh in range(2):
                        h = 2 * hp + hh
                        ps = wkps.tile([128, 512], F32, tag="w5", bufs=3)
                        for j in range(jmax + 1):
                            qp0 = 512 * b
                            lo = max(qp0, 128 * j)
                            w = 512 * b + 512 - lo
                            nc.tensor.matmul(
                                ps[0 : DH + 1, lo - qp0 : 512],
                                v_sb[:, j, h, :],
                                atl[hh][
                                    :,
                                    _off(j) + lo - 128 * j : _off(j) + lo - 128 * j + w,
                                ],
                                start=(j == 0),
                                stop=(j == jmax),
                            )
                        recip = nrm.tile([128, 512], BF16, tag="recip", bufs=3)
                        ust = nrm.tile([128, 512], F32, tag="ust", bufs=3)
                        with nc.allow_low_precision(
                            reason="bf16 reciprocal feeds rank-1 denominator "
                            "broadcast; 0.4% scale noise is within tolerance"
                        ):
                            nc.vector.reciprocal(
                                recip[DH : DH + 1, :], ps[DH : DH + 1, :]
                            )
                        pss.append(ps)
                        recips.append(recip)
                        usts.append(ust)
                    # ust copies after both reciprocals: the DVE stream then
                    # delivers recip[1] before the PE reaches its rank-1
                    for ps, ust in zip(pss, usts):
                        nc.vector.tensor_copy(ust[0:DH, :], ps[0:DH, :])
                    dst = outT_sb[:, hp, b * 512 : (b + 1) * 512]
                    rep = wkps.tile([128, 512], F32, tag="rep", bufs=1)
                    ust2 = nrm.tile([128, 512], F32, tag="ust2", bufs=2)
                    nc.sync.dma_start(out=ust2[DH:128, :], in_=usts[1][0:DH, :])
                    nc.tensor.matmul(
                        rep[0:DH, :],
                        ones_sb[DH : DH + 1, 0:DH],
                        recips[0][DH : DH + 1, :],
                        start=True,
                        stop=True,
                    )
                    nc.tensor.matmul(
                        rep[DH:128, :],
                        ones_sb[DH : DH + 1, DH:128],
                        recips[1][DH : DH + 1, :],
                        start=True,
                        stop=True,
                    )
                    nc.vector.tensor_mul(dst[0:DH, :], usts[0][0:DH, :], rep[0:DH, :])
                    nc.vector.tensor_mul(
                        dst[DH:128, :], ust2[DH:128, :], rep[DH:128, :]
                    )

                # prologue: head-pair 0 needs its full q (scores are key-major:
                # every j reads all query columns >= 128j) but only the first
                # k block; k blocks 1..3 ride along as attention fillers.
                # Chunk-major across 5 concurrent PSUM groups so the PE
                # consumes each x chunk as its DMA lands.
                cq = qkp.tile([128, L], BF16, tag="qT", bufs=2)
                ck = qkp.tile([128, L], BF16, tag="kT", bufs=2)
                pgroups = [(wq_sb, qb, cq) for qb in range(QB)]
                pgroups.append((wk_sb, 0, ck))
                pss = [
                    wkps.tile([128, 512], F32, tag="w5", bufs=3, name=f"pp{i}")
                    for i in range(4)
                ]
                pss.append(
                    wkps.tile([128, 1024], F32, tag="wk", bufs=2, name="pp4")
                )
                for c in range(DC):
                    for ps, (w_sb, qb, _) in zip(pss, pgroups):
                        nc.tensor.matmul(
                            ps[:, 0:512],
                            w_sb[:, c, 0:128],
                            xT_sb[:, c, qb * 512 : (qb + 1) * 512],
                            start=(c == 0),
                            stop=(c == DC - 1),
                        )
                for ps, (_, qb, dst) in zip(pss, pgroups):
                    nc.vector.tensor_copy(
                        dst[:, qb * 512 : (qb + 1) * 512], ps[:, 0:512]
                    )

                if "a" in phases:
                    for hp in range(4):
                        at0 = att.tile([128, ATT_W], BF16, tag="attnT", bufs=2)
                        at1 = att.tile([128, ATT_W], BF16, tag="attnT", bufs=2)
                        atl = (at0, at1)
                        if hp < 3:
                            nq = qkp.tile([128, L], BF16, tag="qT", bufs=2)
                            nk = qkp.tile([128, L], BF16, tag="kT", bufs=2)
                        for b in range(QB):
                            fillers = []
                            if hp == 0:
                                if b < 3:
                                    # head-pair 0's own next k block: block
                                    # b+1's scores need it next iteration
                                    fillers.append(
                                        lambda qb=b + 1: pj_one(wk_sb, 0, qb, ck)
                                    )
                                fillers.append(
                                    lambda qb=b: pj_qk(1, qb, nq, nk)
                                )
                                # v chunks feed av(b-1): one block of lag,
                                # which also rides out the late wv load
                                if b > 0:
                                    fillers += [
                                        (lambda it=4 * (b - 1) + i: pj_v(it))
                                        for i in range(4)
                                    ]
                                if b == 3:
                                    fillers += [
                                        (lambda it=12 + i: pj_v(it))
                                        for i in range(4)
                                    ]
                            elif hp < 3:
                                fillers = [
                                    lambda qb=b, t=hp + 1: pj_qk(t, qb, nq, nk)
                                ]
                            sc_group(hp, b, atl, cq, ck, fillers)
                            if b > 0:
                                # one-block-deep software pipeline: this av's
                                # exp finished while block b's scores ran
                                av_group(hp, b - 1, atl)
                            if hp == 3 and "o" in phases and b > 1:
                                # two blocks of lag so outproj never waits on
                                # the odd-head staging DMA into outT
                                for qt in range(4 * (b - 2), 4 * b - 4):
                                    op_qt(qt)
                        av_group(hp, 3, atl)
                        if hp < 3:
                            cq, ck = nq, nk
                    if "o" in phases:
                        for qt in range(8, 16):
                            op_qt(qt)
                else:
                    # keep projections live when attention is ablated
                    nc.sync.dma_start(
                        out=out_r[:, 0, 0:512], in_=cq[:, 0:512].bitcast(F32)
                    )

        if reps > 0:
            with tc.For_i(0, reps):
                body()
        body()


_CACHE = {}


def _get_runner(reps=0):
    if reps not in _CACHE:
        _CACHE[reps] = build_kernel(reps)
    return _CACHE[reps]


def make_in_maps(x, Wq, Wk, Wv, Wo):
    in_maps = []
    for core in range(N_CORES):
        b, hg = divmod(core, 2)
        sl = slice(hg * DHG, (hg + 1) * DHG)
        in_maps.append(
            {
                "xT": np.ascontiguousarray(np.asarray(x)[b].T.astype(bfloat16)),
                "wqT": np.ascontiguousarray(np.asarray(Wq)[sl, :].T.astype(bfloat16)),
                "wkT": np.ascontiguousarray(np.asarray(Wk)[sl, :].T.astype(bfloat16)),
                "wvT": np.ascontiguousarray(np.asarray(Wv)[sl, :].T.astype(bfloat16)),
                "woT": np.ascontiguousarray(np.asarray(Wo)[:, sl].T.astype(bfloat16)),
            }
        )
    return in_maps


def kernel(x, Wq, Wk, Wv, Wo):
    x = np.asarray(x)
    nc = _get_runner(0)
    in_maps = make_in_maps(x, Wq, Wk, Wv, Wo)
    res = run_bass_kernel_spmd(nc, in_maps, core_ids=list(range(N_CORES)))
    B = x.shape[0]
    out = np.empty((B, L, D), dtype=np.float32)
    for b in range(B):
        out[b] = res.results[2 * b]["out"].astype(np.float32) + res.results[
            2 * b + 1
        ]["out"].astype(np.float32)
    return out


# revision 38
# speedup vs baseline: 1.4953x; 1.4953x over previous
"""Causal multi-head attention (B=4, L=2048, D=1024, H=16) on 8 TRN2 NeuronCores.

Sharding: core = (batch b, head-group hg), b in 0..3, hg in 0..1; each core
computes its batch x 8 heads and the partial out-projection; host sums the two
head-group partials per batch.

Fully-pipelined single-pass structure (vs. the phase-sequential baseline):
  - all operands bf16 (PSUM accumulation f32), which halves SBUF so the
    x / W tiles stay resident and projection matmuls interleave with
    attention as PE "filler" jobs between score tiles:
      hp0 carries the v projection + hp1's q/k projection and its own
      remaining k blocks, hp1/hp2 carry hp+1's q/k projection,
      hp3 carries the out-projection of finished query blocks.
  - scores -> exp in 1024-col PSUM tiles (two banks) to halve ScalarE
    instruction-start overhead; exp runs concurrently with attn@v and the
    interleaved projections; the diagonal causal mask is one DVE multiply
    per key chunk covering both heads (mask broadcast over the head dim).
  - softmax denominator via ones-column in v_aug; per-block lagged
    normalization (cross-engine handoff latency on hardware is ~0.3-0.6us
    per dependent hop, so nothing consumes same-block data): attn@v
    chains + reciprocal + SBUF evac at block b, rank-1 bf16 broadcast
    matmul (ones^T x recip -> PSUM) + normalize multiplies at b+2. Odd
    head rows are DMA-shifted to partitions 64:128 before the multiply.

kernel(**inputs) takes the full unsharded f32 inputs and returns the full
f32 output (device math bf16, host sums the two head-group partials).
"""

import numpy as np
from ml_dtypes import bfloat16

import concourse.bass as bass
import concourse.mybir as mybir
import concourse.tile as tile
from concourse import bacc
from concourse.bass_utils import run_bass_kernel_spmd

F32 = mybir.dt.float32
BF16 = mybir.dt.bfloat16

L = 2048          # sequence length
D = 1024          # model dim
HG = 8            # heads per core
DH = 64           # head dim
DHG = HG * DH     # 512, head-group width
DC = D // 128     # 8 contraction chunks for projections
LT = L // 128     # 16 key-position chunks
QB = L // 512     # 4 query blocks of 512
N_CORES = 8


# column offset of kp-chunk j's storage inside the packed causal attnT buffer
def _off(j):
    return 2048 * j - 128 * (j * (j - 1) // 2)


ATT_W = _off(LT)  # 17408 packed causal columns per head


SKIP = set()


def build_kernel(reps: int = 0, phases: str = "pao"):
    """Build the SPMD Bass program. reps>0 wraps the body in a hardware loop
    (body executed reps+1 times total) for timing."""
    nc = bacc.Bacc()

    xT = nc.dram_tensor("xT", [D, L], BF16, kind="ExternalInput")
    wqT = nc.dram_tensor("wqT", [D, DHG], BF16, kind="ExternalInput")
    wkT = nc.dram_tensor("wkT", [D, DHG], BF16, kind="ExternalInput")
    wvT = nc.dram_tensor("wvT", [D, DHG], BF16, kind="ExternalInput")
    woT = nc.dram_tensor("woT", [DHG, D], BF16, kind="ExternalInput")
    out = nc.dram_tensor("out", [L, D], BF16, kind="ExternalOutput")

    xT_r = xT[:, :].rearrange("(c p) l -> p c l", p=128)
    wqT_r = wqT[:, :].rearrange("(c p) m -> p c m", p=128)
    wkT_r = wkT[:, :].rearrange("(c p) m -> p c m", p=128)
    wvT_r = wvT[:, :].rearrange("(c p) m -> p c m", p=128)
    woT_r = woT[:, :].rearrange("(c p) n -> p c n", p=128)
    out_r = out[:, :].rearrange("(t p) n -> p t n", p=128)

    with tile.TileContext(nc) as tc:
        ctx_body(nc, tc, xT_r, wqT_r, wkT_r, wvT_r, woT_r, out_r, reps, phases)
    nc.compile()
    return nc


def ctx_body(nc, tc, xT_r, wqT_r, wkT_r, wvT_r, woT_r, out_r, reps, phases="pao"):
    from contextlib import ExitStack

    with ExitStack() as es:
        persist = es.enter_context(tc.tile_pool(name="persist", bufs=1))
        mask_sb = persist.tile([128, 128], BF16)  # upper-tri (incl diag) ones
        ones_sb = persist.tile([128, 128], BF16)  # rank-1 denom broadcast
        nc.vector.memset(ones_sb, 1.0)

        # constant setup (outside the timing loop)
        # mask[kp, qp] = 1 where kp <= qp else 0
        nc.gpsimd.memset(mask_sb, 1.0)
        nc.gpsimd.affine_select(
            out=mask_sb,
            in_=mask_sb,
            compare_op=mybir.AluOpType.is_ge,
            fill=0.0,
            base=0,
            pattern=[[1, 128]],
            channel_multiplier=-1,
        )

        def body():
            with ExitStack() as bs:
                glob = bs.enter_context(tc.tile_pool(name="glob", bufs=1))
                qkp = bs.enter_context(tc.tile_pool(name="qkp", bufs=2))
                att = bs.enter_context(tc.tile_pool(name="att", bufs=2))
                nrm = bs.enter_context(tc.tile_pool(name="nrm", bufs=1))
                oev = bs.enter_context(tc.tile_pool(name="oev", bufs=1))
                wkps = bs.enter_context(
                    tc.tile_pool(name="wkps", bufs=1, space="PSUM")
                )

                xT_sb = glob.tile([128, DC, L], BF16)
                wq_sb = glob.tile([128, DC, DHG], BF16)
                wk_sb = glob.tile([128, DC, DHG], BF16)
                wv_sb = glob.tile([128, DC, DHG], BF16)
                wo_sb = glob.tile([128, 4, D], BF16)
                v_sb = glob.tile([128, LT, HG, DH + 1], BF16)
                outT_sb = glob.tile([128, 4, L], BF16)

                # load order tracks first use: x + head-pair-0 slices of
                # wq/wk gate the prologue, the rest trickles in behind
                for c in range(DC):
                    nc.sync.dma_start(out=xT_sb[:, c, :], in_=xT_r[:, c, :])
                    nc.sync.dma_start(
                        out=wq_sb[:, c, 0:256], in_=wqT_r[:, c, 0:256]
                    )
                    nc.sync.dma_start(
                        out=wk_sb[:, c, 0:256], in_=wkT_r[:, c, 0:256]
                    )
                for c in range(DC):
                    nc.sync.dma_start(
                        out=wq_sb[:, c, 256:DHG], in_=wqT_r[:, c, 256:DHG]
                    )
                    nc.sync.dma_start(
                        out=wk_sb[:, c, 256:DHG], in_=wkT_r[:, c, 256:DHG]
                    )
                for c in range(DC):
                    nc.sync.dma_start(out=wv_sb[:, c, :], in_=wvT_r[:, c, :])
                nc.sync.dma_start(out=wo_sb, in_=woT_r)

                # ones column of v_aug (denominator row of attn@v output)
                nc.vector.memset(v_sb[:, :, :, DH : DH + 1], 1.0)

                def pj_one(w_sb, t, qb, dst):
                    # one projection (q or k) for head-pair t, query block qb
                    ps = wkps.tile([128, 512], F32, tag="w5", bufs=3)
                    for c in range(DC):
                        nc.tensor.matmul(
                            ps,
                            w_sb[:, c, t * 128 : (t + 1) * 128],
                            xT_sb[:, c, qb * 512 : (qb + 1) * 512],
                            start=(c == 0),
                            stop=(c == DC - 1),
                        )
                    nc.vector.tensor_copy(dst[:, qb * 512 : (qb + 1) * 512], ps)

                def pj_qk(t, qb, dq, dk):
                    pj_one(wq_sb, t, qb, dq)
                    pj_one(wk_sb, t, qb, dk)

                def pj_v(it):
                    # v projection for kp chunk it (all 8 heads)
                    ps = wkps.tile([128, 512], F32, tag="w5", bufs=3)
                    for c in range(DC):
                        nc.tensor.matmul(
                            ps,
                            xT_sb[:, c, it * 128 : (it + 1) * 128],
                            wv_sb[:, c, :],
                            start=(c == 0),
                            stop=(c == DC - 1),
                        )
                    nc.vector.tensor_copy(
                        v_sb[:, it, :, 0:DH],
                        ps.rearrange("p (h d) -> p h d", h=HG),
                    )

                def op_qt(qt):
                    # out-projection partial for query tile qt; two 512-wide
                    # PSUM tiles from the deeper w5 ring (the 2-deep wk ring
                    # is busy with score tiles when this interleaves into hp3)
                    ot = oev.tile([128, D], BF16, tag="ot", bufs=2)
                    for nh in range(2):
                        ps = wkps.tile([128, 512], F32, tag="w5", bufs=3)
                        for c in range(4):
                            nc.tensor.matmul(
                                ps,
                                outT_sb[:, c, qt * 128 : (qt + 1) * 128],
                                wo_sb[:, c, nh * 512 : (nh + 1) * 512],
                                start=(c == 0),
                                stop=(c == 3),
                            )
                        nc.vector.tensor_copy(
                            ot[:, nh * 512 : (nh + 1) * 512], ps
                        )
                    nc.sync.dma_start(out=out_r[:, qt, :], in_=ot)

                def sc_group(hp, b, atl, cq, ck, fillers):
                    # scores + exp for j-group 4b..4b+3, fillers interleaved
                    fillers = list(fillers)
                    for j in range(4 * b, 4 * b + 4):
                        ncols = L - 128 * j
                        for hh in () if "sc" in SKIP else range(2):
                            p0 = hh * 64
                            for c0 in range(0, ncols, 1024):
                                w = min(1024, ncols - c0)
                                ps = wkps.tile([128, 1024], F32, tag="wk", bufs=2)
                                for s0 in range(0, w, 512):
                                    sw = min(512, w - s0)
                                    q0 = 128 * j + c0 + s0
                                    nc.tensor.matmul(
                                        ps[:, s0 : s0 + sw],
                                        ck[p0 : p0 + 64, j * 128 : (j + 1) * 128],
                                        cq[p0 : p0 + 64, q0 : q0 + sw],
                                        start=True,
                                        stop=True,
                                    )
                                if "exp" not in SKIP:
                                    nc.scalar.activation(
                                        atl[hh][:, _off(j) + c0 : _off(j) + c0 + w],
                                        ps[:, :w],
                                        mybir.ActivationFunctionType.Exp,
                                        scale=0.125,
                                    )
                            if "exp" not in SKIP:
                                # mask the diagonal block of this j (DVE; the
                                # one-block av lag gives it plenty of slack)
                                nc.vector.tensor_mul(
                                    atl[hh][:, _off(j) : _off(j) + 128],
                                    atl[hh][:, _off(j) : _off(j) + 128],
                                    mask_sb,
                                )
                        if fillers:
                            fillers.pop(0)()
                    for f in fillers:
                        f()

                # per-(hp,b) normalization operands awaiting their lagged tail
                pending = {}

                def av_chains(hp, b, atl):
                    # attn @ v_aug chains for qp-block b, both heads, plus the
                    # reciprocal of the denominator row and the SBUF evac of
                    # the un-normalized rows. The rank-1 broadcast + multiply
                    # run two blocks later (norm_tail) so no engine ever waits
                    # on data produced in the same block.
                    if "av" in SKIP:
                        return
                    pss, recips, usts = [], [], []
                    jmax = 4 * b + 3
                    for hh in range(2):
                        h = 2 * hp + hh
                        ps = wkps.tile([128, 512], F32, tag="w5", bufs=3)
                        for j in range(jmax + 1):
                            qp0 = 512 * b
                            lo = max(qp0, 128 * j)
                            w = 512 * b + 512 - lo
                            nc.tensor.matmul(
                                ps[0 : DH + 1, lo - qp0 : 512],
                                v_sb[:, j, h, :],
                                atl[hh][
                                    :,
                                    _off(j) + lo - 128 * j : _off(j) + lo - 128 * j + w,
                                ],
                                start=(j == 0),
                                stop=(j == jmax),
                            )
                        recip = nrm.tile([128, 512], BF16, tag="recip", bufs=5)
                        ust = nrm.tile([128, 512], F32, tag="ust", bufs=5)
                        with nc.allow_low_precision(
                            reason="bf16 reciprocal feeds rank-1 denominator "
                            "broadcast; 0.4% scale noise is within tolerance"
                        ):
                            nc.vector.reciprocal(
                                recip[DH : DH + 1, :], ps[DH : DH + 1, :]
                            )
                        pss.append(ps)
                        recips.append(recip)
                        usts.append(ust)
                    for ps, ust in zip(pss, usts):
                        nc.vector.tensor_copy(ust[0:DH, :], ps[0:DH, :])
                    ust2 = nrm.tile([128, 512], F32, tag="ust2", bufs=3)
                    nc.sync.dma_start(out=ust2[DH:128, :], in_=usts[1][0:DH, :])
                    pending[(hp, b)] = (recips, usts, ust2)

                def norm_tail(hp, b):
                    # rank-1 denominator broadcast (PE) + normalize multiplies
                    # (DVE) for block b, consuming operands prepared two
                    # blocks ago
                    if (hp, b) not in pending:
                        return
                    recips, usts, ust2 = pending.pop((hp, b))
                    dst = outT_sb[:, hp, b * 512 : (b + 1) * 512]
                    rep = wkps.tile([128, 512], F32, tag="rep", bufs=1)
                    nc.tensor.matmul(
                        rep[0:DH, :],
                        ones_sb[DH : DH + 1, 0:DH],
                        recips[0][DH : DH + 1, :],
                        start=True,
                        stop=True,
                    )
                    nc.tensor.matmul(
                        rep[DH:128, :],
                        ones_sb[DH : DH + 1, DH:128],
                        recips[1][DH : DH + 1, :],
                        start=True,
                        stop=True,
                    )
                    nc.vector.tensor_mul(dst[0:DH, :], usts[0][0:DH, :], rep[0:DH, :])
                    nc.vector.tensor_mul(
                        dst[DH:128, :], ust2[DH:128, :], rep[DH:128, :]
                    )

                # prologue: head-pair 0 needs its full q (scores are key-major:
                # every j reads all query columns >= 128j) but only the first
                # k block; k blocks 1..3 ride along as attention fillers.
                # Chunk-major across 5 concurrent PSUM groups so the PE
                # consumes each x chunk as its DMA lands.
                cq = qkp.tile([128, L], BF16, tag="qT", bufs=2)
                ck = qkp.tile([128, L], BF16, tag="kT", bufs=2)
                pgroups = [(wq_sb, qb, cq) for qb in range(QB)]
                pgroups.append((wk_sb, 0, ck))
                pss = [
                    wkps.tile([128, 512], F32, tag="w5", bufs=3, name=f"pp{i}")
                    for i in range(4)
                ]
                pss.append(
                    wkps.tile([128, 1024], F32, tag="wk", bufs=2, name="pp4")
                )
                for c in range(DC):
                    for ps, (w_sb, qb, _) in zip(pss, pgroups):
                        nc.tensor.matmul(
                            ps[:, 0:512],
                            w_sb[:, c, 0:128],
                            xT_sb[:, c, qb * 512 : (qb + 1) * 512],
                            start=(c == 0),
                            stop=(c == DC - 1),
                        )
                for ps, (_, qb, dst) in zip(pss, pgroups):
                    nc.vector.tensor_copy(
                        dst[:, qb * 512 : (qb + 1) * 512], ps[:, 0:512]
                    )

                if "a" in phases:
                    for hp in range(4):
                        at0 = att.tile([128, ATT_W], BF16, tag="attnT", bufs=2)
                        at1 = att.tile([128, ATT_W], BF16, tag="attnT", bufs=2)
                        atl = (at0, at1)
                        if hp < 3:
                            nq = qkp.tile([128, L], BF16, tag="qT", bufs=2)
                            nk = qkp.tile([128, L], BF16, tag="kT", bufs=2)
                        for b in range(QB):
                            fillers = []
                            if hp == 0:
                                if b < 3:
                                    # head-pair 0's own next k block: block
                                    # b+1's scores need it next iteration
                                    fillers.append(
                                        lambda qb=b + 1: pj_one(wk_sb, 0, qb, ck)
                                    )
                                fillers.append(
                                    lambda qb=b: pj_qk(1, qb, nq, nk)
                                )
                                # v chunks feed av(b-1): one block of lag,
                                # which also rides out the late wv load
                                if b > 0:
                                    fillers += [
                                        (lambda it=4 * (b - 1) + i: pj_v(it))
                                        for i in range(4)
                                    ]
                                if b == 3:
                                    fillers += [
                                        (lambda it=12 + i: pj_v(it))
                                        for i in range(4)
                                    ]
                            elif hp < 3:
                                fillers = [
                                    lambda qb=b, t=hp + 1: pj_qk(t, qb, nq, nk)
                                ]
                            sc_group(hp, b, atl, cq, ck, fillers)
                            if b > 0:
                                # one-block lag: this av's exp finished while
                                # block b's scores ran
                                av_chains(hp, b - 1, atl)
                            if b > 1:
                                # two-block lag for the normalization tail
                                norm_tail(hp, b - 2)
                            if hp == 3 and "o" in phases and b == 3:
                                for qt in range(0, 4):
                                    op_qt(qt)
                        av_chains(hp, 3, atl)
                        norm_tail(hp, 2)
                        if hp == 3 and "o" in phases:
                            for qt in range(4, 8):
                                op_qt(qt)
                        norm_tail(hp, 3)
                        if hp < 3:
                            cq, ck = nq, nk
                    if "o" in phases:
                        for qt in range(8, 16):
                            op_qt(qt)
                else:
                    # keep projections live when attention is ablated
                    nc.sync.dma_start(
                        out=out_r[:, 0, 0:512], in_=cq[:, 0:512].bitcast(F32)
                    )

        if reps > 0:
            with tc.For_i(0, reps):
                body()
        body()


_CACHE = {}


def _get_runner(reps=0):
    if reps not in _CACHE:
        _CACHE[reps] = build_kernel(reps)
    return _CACHE[reps]


def make_in_maps(x, Wq, Wk, Wv, Wo):
    in_maps = []
    for core in range(N_CORES):
        b, hg = divmod(core, 2)
        sl = slice(hg * DHG, (hg + 1) * DHG)
        in_maps.append(
            {
                "xT": np.ascontiguousarray(np.asarray(x)[b].T.astype(bfloat16)),
                "wqT": np.ascontiguousarray(np.asarray(Wq)[sl, :].T.astype(bfloat16)),
                "wkT": np.ascontiguousarray(np.asarray(Wk)[sl, :].T.astype(bfloat16)),
                "wvT": np.ascontiguousarray(np.asarray(Wv)[sl, :].T.astype(bfloat16)),
                "woT": np.ascontiguousarray(np.asarray(Wo)[:, sl].T.astype(bfloat16)),
            }
        )
    return in_maps


def kernel(x, Wq, Wk, Wv, Wo):
    x = np.asarray(x)
    nc = _get_runner(0)
    in_maps = make_in_maps(x, Wq, Wk, Wv, Wo)
    res = run_bass_kernel_spmd(nc, in_maps, core_ids=list(range(N_CORES)))
    B = x.shape[0]
    out = np.empty((B, L, D), dtype=np.float32)
    for b in range(B):
        out[b] = res.results[2 * b]["out"].astype(np.float32) + res.results[
            2 * b + 1
        ]["out"].astype(np.float32)
    return out
